# revision 9
# baseline (speedup 1.0000x reference)
"""GCNConv layer on 8 trn2 NeuronCores (Bass/Tile).

out = scatter_add_{e:(s,d)} dis[d]*dis[s]*(x[s] @ W.T + b), edges incl. self-loops,
dis = 1/sqrt(1 + indegree).

Strategy (1D destination partitioning):
  Host (index-only work): append self-loops, count degrees / build CSR rowptr,
  load-balance destinations into 784 tiles of 128 slots (snake over
  degree-sorted nodes), sort each tile's edges by source-table row, pad each
  (tile, segment) run to a per-position count shared by all 8 cores so one
  SPMD program fits every core.  Stage x^T / W as bf16, rowptr views as f32,
  per-edge gather indices as wrapped int16, local-dest slots as fp16.
  Device phase 1: h = x@W.T (+b) per 128-node tile on PE, scale rows by
  dis (rowptr diffs -> reciprocal -> sqrt), write a bf16 g-table
  [rows, 128] (64 data + 64 zero cols) to DRAM.
  Device phase 2: per group of dest tiles and per 32768-row table segment,
  one bulk dma_gather (gpsimd mlp-library ucode) pulls the source rows;
  one-hot S = (iota_group == local_dest) built on DVE per span; PE
  accumulates S^T @ G into PSUM per dest tile; scale by dest dis; write out.
  Host: unpermute rows.
"""
import numpy as np
import ml_dtypes

import concourse.bass as bass
import concourse.mybir as mybir
from concourse.bass_utils import run_bass_kernel_spmd
from concourse.tile import TileContext, add_dep_helper
from concourse.library_config import mlp
from concourse.library_overlay import lower_extended_insts

BF16 = ml_dtypes.bfloat16
FP16 = np.float16

N_NODES = 100000
D_IN = 128
D_OUT = 64
P = 128
ELEMW = 128          # g-table row width (bf16) = 256B, dma_gather granule

LAST_RESULT = None


def default_cfg():
    return dict(
        n_real=N_NODES,
        n_cores=8,
        ntiles=784,      # destination tiles (128 slots each)
        cn_tiles=64,     # node-tiles per phase-1 x-chunk
        group=7,         # dest tiles per phase-2 gather group (98 = 14*7)
        seg_rows=32768,  # table rows addressable by one int16 gather segment
    )


# ---------------------------------------------------------------- host prep
def preprocess(x, W, b, edge_index, cfg):
    n_real = cfg["n_real"]
    ntiles = cfg["ntiles"]
    n_cores = cfg["n_cores"]
    npad = ntiles * P
    tpc = ntiles // n_cores
    SEG = cfg["seg_rows"]
    nseg = (npad + SEG - 1) // SEG
    group = cfg["group"]
    assert tpc % group == 0
    ngroups = tpc // group
    assert npad > n_real + nseg, "need pad slots incl. one zero row per segment"

    fa = np.asarray(edge_index[0], dtype=np.int64)
    ta = np.asarray(edge_index[1], dtype=np.int64)
    loops = np.arange(n_real, dtype=np.int64)
    fa2 = np.concatenate([fa, loops])
    ta2 = np.concatenate([ta, loops])

    deg = np.bincount(ta2, minlength=n_real).astype(np.int64)  # incl. self-loop
    rowptr = np.zeros(n_real + 1, dtype=np.int64)
    np.cumsum(deg, out=rowptr[1:])

    # --- destination balancing: snake over degree-sorted nodes -> tiles
    order = np.argsort(-deg, kind="stable")
    tile_of = np.empty(n_real, dtype=np.int64)
    slot_of = np.empty(n_real, dtype=np.int64)
    node_at = np.full((ntiles, P), -1, dtype=np.int64)
    pos = 0
    rnd = 0
    counts = np.zeros(ntiles, dtype=np.int64)
    while pos < n_real:
        k = min(ntiles, n_real - pos)
        tiles = np.arange(k) if rnd % 2 == 0 else (ntiles - 1 - np.arange(k))
        nodes = order[pos:pos + k]
        tile_of[nodes] = tiles
        slot_of[nodes] = counts[tiles]
        node_at[tiles, counts[tiles]] = nodes
        counts[tiles] += 1
        pos += k
        rnd += 1

    # --- source-table placement: reserve one zero row per segment
    zrows = np.array([min(npad, (s + 1) * SEG) - 1 for s in range(nseg)],
                     dtype=np.int64)
    is_free = np.ones(npad, dtype=bool)
    is_free[zrows] = False
    free_rows = np.nonzero(is_free)[0]
    node_of_row = np.full(npad, -1, dtype=np.int64)
    node_of_row[free_rows[:n_real]] = np.arange(n_real)
    row_of_node = np.empty(n_real, dtype=np.int64)
    row_of_node[:] = free_rows[:n_real]

    # --- phase-1 x-chunk geometry: row r = (cn*P)*b + TC_b*p + j
    cn = cfg["cn_tiles"]
    nfull = ntiles // cn
    rem_tiles = ntiles - nfull * cn
    chunk_tiles = [cn] * nfull + ([rem_tiles] if rem_tiles else [])
    rows = np.arange(npad, dtype=np.int64)
    bidx = np.minimum(rows // (cn * P), nfull)
    w = rows - bidx * (cn * P)
    tcb = np.where(bidx < nfull, cn, max(rem_tiles, 1))
    pr = w // tcb
    jr = w - pr * tcb
    # x^T column that must hold the node living at table row r:
    xcol_of_row = (cn * P) * bidx + P * jr + pr
    colnode = np.full(npad, -1, dtype=np.int64)
    colnode[xcol_of_row] = node_of_row[rows]

    # --- per-edge keys, sorted by (dest tile, segment, row)
    e_tile = tile_of[ta2]
    e_row = row_of_node[fa2]
    e_seg = e_row // SEG
    e_rel = e_row - e_seg * SEG
    srt = np.lexsort((e_rel, e_seg, e_tile))
    e_tile = e_tile[srt]
    e_seg = e_seg[srt]
    e_rel = e_rel[srt]
    e_slot = slot_of[ta2][srt]

    # counts per (tile, seg); uniform padded length per (tile position, seg)
    cnt = np.zeros((ntiles, nseg), dtype=np.int64)
    np.add.at(cnt, (e_tile, e_seg), 1)
    L = cnt.reshape(n_cores, tpc, nseg).max(axis=0)  # [tpc, nseg]

    # group totals padded to whole chunks; uniform per segment so the device
    # program needs only nseg distinct num_idxs registers
    Lg = L.reshape(ngroups, group, nseg).sum(axis=1)         # [ngroups, nseg]
    Cgs = ((Lg + P - 1) // P).astype(np.int64)               # chunks
    Cgs[:] = Cgs.max(axis=0, keepdims=True)
    NGs = Cgs * P

    zrel = zrows - (zrows // SEG) * SEG                      # per-seg zero row

    # --- build per-core gather index + local-dest arrays
    tstart = np.zeros(ntiles * nseg + 1, dtype=np.int64)
    np.cumsum(cnt.reshape(-1), out=tstart[1:])
    tot16 = int(NGs.sum()) // 16
    totC = int(Cgs.sum())
    per_core = []
    for c in range(n_cores):
        gidx = np.zeros((P, tot16), dtype=np.int16)
        tloc = np.zeros((P, totC), dtype=FP16)
        col16 = 0
        colC = 0
        for g in range(ngroups):
            for s in range(nseg):
                NG = int(NGs[g, s])
                if NG == 0:
                    continue
                flat_idx = np.full(NG, zrel[s], dtype=np.int16)
                flat_tl = np.zeros(NG, dtype=np.float32)
                o = 0
                for t in range(group):
                    tl = g * group + t
                    gt = c * tpc + tl
                    n_e = int(cnt[gt, s])
                    s0 = tstart[gt * nseg + s]
                    flat_idx[o:o + n_e] = e_rel[s0:s0 + n_e]
                    flat_tl[o:o + n_e] = e_slot[s0:s0 + n_e] + t * P
                    o += int(L[tl, s])
                # wrap indices in 16 partitions, replicated across 8 q7 cores
                wrapped = flat_idx.reshape(NG // 16, 16).T      # [16, NG/16]
                gidx[:, col16:col16 + NG // 16] = np.tile(wrapped, (8, 1))
                # local-dest values at [p, chunk]
                tloc[:, colC:colC + NG // P] = (
                    flat_tl.reshape(NG // P, P).T.astype(FP16))
                col16 += NG // 16
                colC += NG // P
        per_core.append(dict(gidx=gidx, tloc=tloc))

    # --- degA (phase-1 x-column layout [P, ntiles])
    q = np.arange(ntiles, dtype=np.int64)
    qb = np.minimum(q // cn, nfull)
    qj = q - qb * cn
    colnode2 = colnode[((cn * P) * qb + P * qj)[None, :]
                       + np.arange(P, dtype=np.int64)[:, None]]
    degA_lo = np.zeros((P, ntiles), dtype=np.float32)
    degA_hi = np.zeros((P, ntiles), dtype=np.float32)
    realA = colnode2 >= 0
    degA_lo[realA] = rowptr[colnode2[realA]]
    degA_hi[realA] = rowptr[colnode2[realA] + 1]
    degA_hi[~realA] = degA_lo[~realA] + 1.0e30   # pads/zero rows: dis ~ 0

    # --- degB (dest layout, per core [P, tpc])
    lo = np.zeros((P, ntiles), dtype=np.float32)
    hi = np.ones((P, ntiles), dtype=np.float32)
    na = node_at.T
    realb = na >= 0
    lo[realb] = rowptr[na[realb]]
    hi[realb] = rowptr[na[realb] + 1]
    for c in range(n_cores):
        sl = slice(c * tpc, (c + 1) * tpc)
        per_core[c]["degB_lo"] = np.ascontiguousarray(lo[:, sl])
        per_core[c]["degB_hi"] = np.ascontiguousarray(hi[:, sl])

    # --- staged tensors
    xf = np.asarray(x, dtype=np.float32)
    xT = np.zeros((P, npad), dtype=BF16)
    valid = colnode >= 0
    xT[:, valid] = xf[colnode[valid]].T.astype(BF16)
    Wt = np.zeros((P, ELEMW), dtype=BF16)
    Wt[:, :D_OUT] = np.asarray(W, np.float32).T.astype(BF16)
    brow = np.zeros((1, ELEMW), dtype=BF16)
    brow[0, :D_OUT] = np.asarray(b, np.float32).astype(BF16)
    iotaG = np.tile(np.arange(group * P, dtype=FP16)[None, :], (P, 1))

    shared = dict(xT=xT, Wt=Wt, degA_lo=degA_lo, degA_hi=degA_hi, iotaG=iotaG)
    asm = dict(node_at=node_at, tpc=tpc, n_real=n_real,
               chunk_tiles=chunk_tiles,
               L=L, Cgs=Cgs, nseg=nseg, ngroups=ngroups, group=group,
               zrel=zrel, npad=npad)
    return shared, per_core, brow, asm


# ---------------------------------------------------------------- wait splits
def split_excess_waits(nc, max_waits=1):
    """This walrus encodes at most one sync-wait per instruction: peel extras
    onto single-wait nops inserted before it on the same engine."""
    for bb in nc.main_func.blocks:
        insts = bb.instructions
        i = 0
        while i < len(insts):
            ins = insts[i]
            si = ins.sync_info
            if si is not None and si.on_wait and len(si.on_wait) > max_waits:
                extra = list(si.on_wait[max_waits:])
                keep = list(si.on_wait[:max_waits])
                carriers = []
                for w_ in extra:
                    nop = nc.engines[ins.engine].nop(hint="wsplit", nofuse=True).ins
                    nop.sync_info = mybir.SyncInfo(on_wait=[w_], on_update=[])
                    for bb2 in nc.main_func.blocks:
                        if nop in bb2.instructions:
                            bb2.instructions.remove(nop)
                    carriers.append(nop)
                si.on_wait = keep
                for k, nop in enumerate(carriers):
                    insts.insert(i + k, nop)
                i += len(carriers)
            i += 1


# ---------------------------------------------------------------- device prog
def build_program(cfg, asm, bias_on):
    ntiles = cfg["ntiles"]
    n_cores = cfg["n_cores"]
    tpc = ntiles // n_cores
    npad = asm["npad"]
    cn = cfg["cn_tiles"]
    SEG = cfg["seg_rows"]
    nseg = asm["nseg"]
    ngroups = asm["ngroups"]
    group = asm["group"]
    L = asm["L"]
    Cgs = asm["Cgs"]
    chunk_tiles = asm["chunk_tiles"]
    f32 = mybir.dt.float32
    bf16 = mybir.dt.bfloat16
    fp16 = mybir.dt.float16
    i16 = mybir.dt.int16
    EQ = mybir.AluOpType.is_equal
    MUL = mybir.AluOpType.mult
    SUB = mybir.AluOpType.subtract

    tot16 = int((Cgs * P).sum()) // 16
    totC = int(Cgs.sum())

    nc = bass.Bass()
    xT = nc.declare_dram_parameter("xT", [P, npad], bf16, isOutput=False)
    Wt = nc.declare_dram_parameter("Wt", [P, ELEMW], bf16, isOutput=False)
    degA_lo = nc.declare_dram_parameter("degA_lo", [P, ntiles], f32, isOutput=False)
    degA_hi = nc.declare_dram_parameter("degA_hi", [P, ntiles], f32, isOutput=False)
    iotaG = nc.declare_dram_parameter("iotaG", [P, group * P], fp16, isOutput=False)
    degB_lo = nc.declare_dram_parameter("degB_lo", [P, tpc], f32, isOutput=False)
    degB_hi = nc.declare_dram_parameter("degB_hi", [P, tpc], f32, isOutput=False)
    gidx = nc.declare_dram_parameter("gidx", [P, tot16], i16, isOutput=False)
    tloc = nc.declare_dram_parameter("tloc", [P, totC], fp16, isOutput=False)
    if bias_on:
        brow_p = nc.declare_dram_parameter("brow", [1, ELEMW], bf16, isOutput=False)
    outp = nc.declare_dram_parameter("outp", [tpc * P, D_OUT], f32, isOutput=True)
    g_table = nc.dram_tensor("g_table", [npad, ELEMW], bf16)

    nc.gpsimd.load_library(mlp)
    ng_regs = {}
    for s in range(nseg):
        C = int(Cgs[0, s])
        if C > 0:
            ng_regs[s] = nc.gpsimd.to_reg(C * P)

    with TileContext(nc) as tc:
        with (
            tc.tile_pool(name="const", bufs=1) as cpool,
            tc.tile_pool(name="xin", bufs=2) as xpool,
            tc.tile_pool(name="gw", bufs=2) as gwpool,
            tc.tile_pool(name="ps1", bufs=4, space="PSUM") as ps1,
            tc.tile_pool(name="gather", bufs=2) as gpool,
            tc.tile_pool(name="sel", bufs=3) as spool,
            tc.tile_pool(name="ps2", bufs=4, space="PSUM") as ps2,
            tc.tile_pool(name="outb", bufs=2) as opool,
        ):
            # ---- constants
            Wt_sb = cpool.tile([P, ELEMW], bf16)
            nc.sync.dma_start(out=Wt_sb[:], in_=Wt[:])
            iotaG_sb = cpool.tile([P, group * P], fp16)
            nc.sync.dma_start(out=iotaG_sb[:], in_=iotaG[:])
            if bias_on:
                ones_sb = cpool.tile([1, P], bf16)
                nc.vector.memset(ones_sb[:], 1.0)
                brow_sb = cpool.tile([1, ELEMW], bf16)
                nc.sync.dma_start(out=brow_sb[:], in_=brow_p[:])

            dAl = cpool.tile([P, ntiles], f32)
            dAh = cpool.tile([P, ntiles], f32)
            nc.sync.dma_start(out=dAl[:], in_=degA_lo[:])
            nc.sync.dma_start(out=dAh[:], in_=degA_hi[:])
            disA = cpool.tile([P, ntiles], f32)
            nc.vector.tensor_tensor(out=disA[:], in0=dAh[:], in1=dAl[:], op=SUB)
            nc.vector.reciprocal(out=disA[:], in_=disA[:])
            nc.scalar.activation(out=disA[:], in_=disA[:],
                                 func=mybir.ActivationFunctionType.Sqrt)

            dBl = cpool.tile([P, tpc], f32)
            dBh = cpool.tile([P, tpc], f32)
            nc.sync.dma_start(out=dBl[:], in_=degB_lo[:])
            nc.sync.dma_start(out=dBh[:], in_=degB_hi[:])
            disB = cpool.tile([P, tpc], f32)
            nc.vector.tensor_tensor(out=disB[:], in0=dBh[:], in1=dBl[:], op=SUB)
            nc.vector.reciprocal(out=disB[:], in_=disB[:])
            nc.scalar.activation(out=disB[:], in_=disB[:],
                                 func=mybir.ActivationFunctionType.Sqrt)

            gidx_sb = cpool.tile([P, tot16], i16)
            nc.sync.dma_start(out=gidx_sb[:], in_=gidx[:])
            tloc_sb = cpool.tile([P, totC], fp16)
            nc.sync.dma_start(out=tloc_sb[:], in_=tloc[:])

            # ---- phase 1: write g-table
            table_writes = []
            q = 0
            off = 0
            roff = 0
            for tcb in chunk_tiles:
                xt = xpool.tile([P, cn * P], bf16, tag="xt")
                nc.sync.dma_start(out=xt[:, :tcb * P], in_=xT[:, off:off + tcb * P])
                gbig = gwpool.tile([P, cn * ELEMW], bf16, tag="gbig")
                for j in range(tcb):
                    hp = ps1.tile([P, ELEMW], f32)
                    nc.tensor.matmul(out=hp[:], lhsT=xt[:, j * P:(j + 1) * P],
                                     rhs=Wt_sb[:], start=True, stop=not bias_on)
                    if bias_on:
                        nc.tensor.matmul(out=hp[:], lhsT=ones_sb[:], rhs=brow_sb[:],
                                         start=False, stop=True)
                    nc.vector.tensor_scalar(
                        out=gbig[:, j * ELEMW:(j + 1) * ELEMW], in0=hp[:],
                        scalar1=disA[:, q:q + 1], scalar2=None, op0=MUL)
                    q += 1
                wr = nc.sync.dma_start(out=g_table[roff:roff + tcb * P, :],
                                       in_=gbig[:, :tcb * ELEMW])
                table_writes.append(wr)
                off += tcb * P
                roff += tcb * P

            # ---- phase 2: segmented bulk gathers + one-hot matmul
            col16 = 0
            colC_base = np.zeros((ngroups, nseg), dtype=np.int64)
            acc = 0
            for g in range(ngroups):
                for s in range(nseg):
                    colC_base[g, s] = acc
                    acc += int(Cgs[g, s])

            orow = 0
            for g in range(ngroups):
                gts = {}
                for s in range(nseg):
                    C = int(Cgs[g, s])
                    if C == 0:
                        continue
                    NG = C * P
                    gb = gpool.tile([P, C, ELEMW], bf16, tag=f"gb{s}")
                    seg_base = s * SEG
                    seg_rows = min(SEG, npad - seg_base)
                    ga = nc.gpsimd.dma_gather(
                        gb[:, :, :], g_table[seg_base:seg_base + seg_rows, :],
                        gidx_sb[:, col16:col16 + NG // 16], NG, ng_regs[s],
                        ELEMW, single_packet=False)
                    for w_ in table_writes:
                        add_dep_helper(ga.ins, w_.ins, reason="gather after table")
                    gts[s] = gb
                    col16 += NG // 16
                ob = opool.tile([P, group * D_OUT], f32, tag="ob")
                for t in range(group):
                    tl = g * group + t
                    mms = []   # (S_tile, k, s, c)
                    for s in range(nseg):
                        Lt = int(L[tl, s])
                        if Lt == 0:
                            continue
                        o0 = int(L[g * group:tl, s].sum())
                        c0 = o0 // P
                        c1 = (o0 + Lt - 1) // P
                        nsp = c1 - c0 + 1
                        S = spool.tile([P, nsp, P], bf16, tag="S")
                        cb = int(colC_base[g, s])
                        nc.vector.tensor_tensor(
                            out=S[:, :, :],
                            in0=iotaG_sb[:, t * P:(t + 1) * P][:, None, :]
                                .to_broadcast([P, nsp, P]),
                            in1=tloc_sb[:, cb + c0:cb + c1 + 1][:, :, None]
                                .to_broadcast([P, nsp, P]),
                            op=EQ)
                        for k, c in enumerate(range(c0, c1 + 1)):
                            mms.append((S, k, s, c))
                    pp = ps2.tile([P, D_OUT], f32)
                    if not mms:
                        nc.vector.memset(ob[:, t * D_OUT:(t + 1) * D_OUT], 0.0)
                        continue
                    for i, (S, k, s, c) in enumerate(mms):
                        nc.tensor.matmul(out=pp[:], lhsT=S[:, k, :],
                                         rhs=gts[s][:, c, 0:D_OUT],
                                         start=(i == 0), stop=(i == len(mms) - 1))
                    nc.vector.tensor_scalar(
                        out=ob[:, t * D_OUT:(t + 1) * D_OUT], in0=pp[:],
                        scalar1=disB[:, tl:tl + 1], scalar2=None, op0=MUL)
                nc.sync.dma_start(out=outp[orow:orow + group * P, :], in_=ob[:])
                orow += group * P

    split_excess_waits(nc)
    lower_extended_insts(nc)
    return nc


# ---------------------------------------------------------------- entry point
def kernel(x, W, b, edge_index):
    cfg = default_cfg()
    shared, per_core, brow, asm = preprocess(x, W, b, edge_index, cfg)
    bias_on = bool(np.any(np.asarray(b) != 0))
    nc = build_program(cfg, asm, bias_on)

    in_maps = []
    for c in range(cfg["n_cores"]):
        m = dict(shared)
        m.update(per_core[c])
        if bias_on:
            m["brow"] = brow
        in_maps.append(m)
    res = run_bass_kernel_spmd(nc, in_maps, list(range(cfg["n_cores"])))
    global LAST_RESULT
    LAST_RESULT = res

    return assemble(res.results, asm, cfg)


def assemble(results, asm, cfg):
    n_real = asm["n_real"]
    tpc = asm["tpc"]
    node_at = asm["node_at"]
    group = asm["group"]
    ngroups = asm["ngroups"]
    out = np.zeros((n_real, D_OUT), dtype=np.float32)
    for c in range(cfg["n_cores"]):
        r = results[c]["outp"]  # [tpc*P, 64]
        for g in range(ngroups):
            blk = r[g * group * P:(g + 1) * group * P].reshape(P, group, D_OUT)
            for t in range(group):
                nd = node_at[c * tpc + g * group + t]
                ok = nd >= 0
                out[nd[ok]] = blk[ok, t]
    return out


# revision 12
# speedup vs baseline: 2.2777x; 2.2777x over previous
"""GCNConv layer on 8 trn2 NeuronCores (Bass/Tile).

out = scatter_add_{e:(s,d)} dis[d]*dis[s]*(x[s] @ W.T + b), edges incl. self-loops,
dis = 1/sqrt(1 + indegree).

Strategy (1D destination partitioning):
  Host (index-only work): append self-loops, count degrees / build CSR rowptr,
  load-balance destinations into 784 tiles of 128 slots (snake over
  degree-sorted nodes), sort each tile's edges by source-table row, pad each
  (tile, segment) run to a per-position count shared by all 8 cores so one
  SPMD program fits every core.  Stage x^T / W as bf16, rowptr views as f32,
  per-edge gather indices as wrapped int16, local-dest slots as fp16.
  Device phase 1: h = x@W.T (+b) per 128-node tile on PE, scale rows by
  dis (rowptr diffs -> reciprocal -> sqrt), write a bf16 g-table
  [rows, 128] (64 data + 64 zero cols) to DRAM.
  Device phase 2: per group of dest tiles and per 32768-row table segment,
  one bulk dma_gather (gpsimd mlp-library ucode) pulls the source rows;
  one-hot S = (iota_group == local_dest) built on DVE per span; PE
  accumulates S^T @ G into PSUM per dest tile; scale by dest dis; write out.
  Host: unpermute rows.
"""
import numpy as np
import ml_dtypes

import concourse.bass as bass
import concourse.mybir as mybir
from concourse.bass_utils import run_bass_kernel_spmd
from concourse.tile import TileContext, add_dep_helper
from concourse.library_config import mlp
from concourse.library_overlay import lower_extended_insts

BF16 = ml_dtypes.bfloat16
FP16 = np.float16

N_NODES = 100000
D_IN = 128
D_OUT = 64
P = 128
ELEMW = 128          # g-table row width (bf16) = 256B, dma_gather granule

LAST_RESULT = None


def default_cfg():
    return dict(
        n_real=N_NODES,
        n_cores=8,
        ntiles=784,      # destination tiles (128 slots each)
        cn_tiles=64,     # node-tiles per phase-1 x-chunk
        group=7,         # dest tiles per phase-2 gather group (98 = 14*7)
        seg_rows=32768,  # table rows addressable by one int16 gather segment
    )


# ---------------------------------------------------------------- host prep
def preprocess(x, W, b, edge_index, cfg):
    n_real = cfg["n_real"]
    ntiles = cfg["ntiles"]
    n_cores = cfg["n_cores"]
    npad = ntiles * P
    tpc = ntiles // n_cores
    SEG = cfg["seg_rows"]
    nseg = (npad + SEG - 1) // SEG
    group = cfg["group"]
    assert tpc % group == 0
    ngroups = tpc // group
    assert npad > n_real + nseg, "need pad slots incl. one zero row per segment"

    fa = np.asarray(edge_index[0], dtype=np.int64)
    ta = np.asarray(edge_index[1], dtype=np.int64)
    loops = np.arange(n_real, dtype=np.int64)
    fa2 = np.concatenate([fa, loops])
    ta2 = np.concatenate([ta, loops])

    deg = np.bincount(ta2, minlength=n_real).astype(np.int64)  # incl. self-loop
    rowptr = np.zeros(n_real + 1, dtype=np.int64)
    np.cumsum(deg, out=rowptr[1:])

    # --- destination balancing: snake over degree-sorted nodes -> tiles
    order = np.argsort(-deg, kind="stable")
    tile_of = np.empty(n_real, dtype=np.int64)
    slot_of = np.empty(n_real, dtype=np.int64)
    node_at = np.full((ntiles, P), -1, dtype=np.int64)
    pos = 0
    rnd = 0
    counts = np.zeros(ntiles, dtype=np.int64)
    while pos < n_real:
        k = min(ntiles, n_real - pos)
        tiles = np.arange(k) if rnd % 2 == 0 else (ntiles - 1 - np.arange(k))
        nodes = order[pos:pos + k]
        tile_of[nodes] = tiles
        slot_of[nodes] = counts[tiles]
        node_at[tiles, counts[tiles]] = nodes
        counts[tiles] += 1
        pos += k
        rnd += 1

    # --- source-table placement: reserve one zero row per segment
    zrows = np.array([min(npad, (s + 1) * SEG) - 1 for s in range(nseg)],
                     dtype=np.int64)
    is_free = np.ones(npad, dtype=bool)
    is_free[zrows] = False
    free_rows = np.nonzero(is_free)[0]
    node_of_row = np.full(npad, -1, dtype=np.int64)
    node_of_row[free_rows[:n_real]] = np.arange(n_real)
    row_of_node = np.empty(n_real, dtype=np.int64)
    row_of_node[:] = free_rows[:n_real]

    # --- phase-1 x-chunk geometry: row r = (cn*P)*b + TC_b*p + j
    cn = cfg["cn_tiles"]
    nfull = ntiles // cn
    rem_tiles = ntiles - nfull * cn
    chunk_tiles = [cn] * nfull + ([rem_tiles] if rem_tiles else [])
    rows = np.arange(npad, dtype=np.int64)
    bidx = np.minimum(rows // (cn * P), nfull)
    w = rows - bidx * (cn * P)
    tcb = np.where(bidx < nfull, cn, max(rem_tiles, 1))
    pr = w // tcb
    jr = w - pr * tcb
    # x^T column that must hold the node living at table row r:
    xcol_of_row = (cn * P) * bidx + P * jr + pr
    colnode = np.full(npad, -1, dtype=np.int64)
    colnode[xcol_of_row] = node_of_row[rows]

    # --- per-edge keys, sorted by (dest tile, segment, row)
    e_tile = tile_of[ta2]
    e_row = row_of_node[fa2]
    e_seg = e_row // SEG
    e_rel = e_row - e_seg * SEG
    srt = np.lexsort((e_rel, e_seg, e_tile))
    e_tile = e_tile[srt]
    e_seg = e_seg[srt]
    e_rel = e_rel[srt]
    e_slot = slot_of[ta2][srt]

    # counts per (tile, seg); uniform padded length per (tile position, seg)
    cnt = np.zeros((ntiles, nseg), dtype=np.int64)
    np.add.at(cnt, (e_tile, e_seg), 1)
    L = cnt.reshape(n_cores, tpc, nseg).max(axis=0)  # [tpc, nseg]

    # group totals padded to whole chunks; uniform per segment so the device
    # program needs only nseg distinct num_idxs registers
    Lg = L.reshape(ngroups, group, nseg).sum(axis=1)         # [ngroups, nseg]
    Cgs = ((Lg + P - 1) // P).astype(np.int64)               # chunks
    Cgs[:] = Cgs.max(axis=0, keepdims=True)
    NGs = Cgs * P

    zrel = zrows - (zrows // SEG) * SEG                      # per-seg zero row

    # --- build per-core gather index + local-dest arrays
    tstart = np.zeros(ntiles * nseg + 1, dtype=np.int64)
    np.cumsum(cnt.reshape(-1), out=tstart[1:])
    tot16 = int(NGs.sum()) // 16
    totC = int(Cgs.sum())
    per_core = []
    for c in range(n_cores):
        gidx = np.zeros((P, tot16), dtype=np.int16)
        tloc = np.zeros((P, totC), dtype=FP16)
        col16 = 0
        colC = 0
        for g in range(ngroups):
            for s in range(nseg):
                NG = int(NGs[g, s])
                if NG == 0:
                    continue
                flat_idx = np.full(NG, zrel[s], dtype=np.int16)
                flat_tl = np.zeros(NG, dtype=np.float32)
                o = 0
                for t in range(group):
                    tl = g * group + t
                    gt = c * tpc + tl
                    n_e = int(cnt[gt, s])
                    s0 = tstart[gt * nseg + s]
                    flat_idx[o:o + n_e] = e_rel[s0:s0 + n_e]
                    flat_tl[o:o + n_e] = e_slot[s0:s0 + n_e] + t * P
                    o += int(L[tl, s])
                # wrap indices in 16 partitions, replicated across 8 q7 cores
                wrapped = flat_idx.reshape(NG // 16, 16).T      # [16, NG/16]
                gidx[:, col16:col16 + NG // 16] = np.tile(wrapped, (8, 1))
                # local-dest values at [p, chunk]
                tloc[:, colC:colC + NG // P] = (
                    flat_tl.reshape(NG // P, P).T.astype(FP16))
                col16 += NG // 16
                colC += NG // P
        per_core.append(dict(gidx=gidx, tloc=tloc))

    # --- degA (phase-1 x-column layout [P, ntiles])
    q = np.arange(ntiles, dtype=np.int64)
    qb = np.minimum(q // cn, nfull)
    qj = q - qb * cn
    colnode2 = colnode[((cn * P) * qb + P * qj)[None, :]
                       + np.arange(P, dtype=np.int64)[:, None]]
    degA_lo = np.zeros((P, ntiles), dtype=np.float32)
    degA_hi = np.zeros((P, ntiles), dtype=np.float32)
    realA = colnode2 >= 0
    degA_lo[realA] = rowptr[colnode2[realA]]
    degA_hi[realA] = rowptr[colnode2[realA] + 1]
    degA_hi[~realA] = degA_lo[~realA] + 1.0e30   # pads/zero rows: dis ~ 0

    # --- degB (dest layout, per core [P, tpc])
    lo = np.zeros((P, ntiles), dtype=np.float32)
    hi = np.ones((P, ntiles), dtype=np.float32)
    na = node_at.T
    realb = na >= 0
    lo[realb] = rowptr[na[realb]]
    hi[realb] = rowptr[na[realb] + 1]
    for c in range(n_cores):
        sl = slice(c * tpc, (c + 1) * tpc)
        per_core[c]["degB_lo"] = np.ascontiguousarray(lo[:, sl])
        per_core[c]["degB_hi"] = np.ascontiguousarray(hi[:, sl])

    # --- staged tensors
    xf = np.asarray(x, dtype=np.float32)
    xT = np.zeros((P, npad), dtype=BF16)
    valid = colnode >= 0
    xT[:, valid] = xf[colnode[valid]].T.astype(BF16)
    Wt = np.zeros((P, ELEMW), dtype=BF16)
    Wt[:, :D_OUT] = np.asarray(W, np.float32).T.astype(BF16)
    brow = np.zeros((1, ELEMW), dtype=BF16)
    brow[0, :D_OUT] = np.asarray(b, np.float32).astype(BF16)
    iotaG = np.tile(np.arange(group * P, dtype=FP16)[None, :], (P, 1))

    shared = dict(xT=xT, Wt=Wt, degA_lo=degA_lo, degA_hi=degA_hi, iotaG=iotaG)
    asm = dict(node_at=node_at, tpc=tpc, n_real=n_real,
               chunk_tiles=chunk_tiles,
               L=L, Cgs=Cgs, nseg=nseg, ngroups=ngroups, group=group,
               zrel=zrel, npad=npad)
    return shared, per_core, brow, asm


# ---------------------------------------------------------------- wait splits
def split_excess_waits(nc, max_waits=1):
    """This walrus encodes at most one sync-wait per instruction: peel extras
    onto single-wait nops inserted before it on the same engine."""
    for bb in nc.main_func.blocks:
        insts = bb.instructions
        i = 0
        while i < len(insts):
            ins = insts[i]
            si = ins.sync_info
            if si is not None and si.on_wait and len(si.on_wait) > max_waits:
                extra = list(si.on_wait[max_waits:])
                keep = list(si.on_wait[:max_waits])
                carriers = []
                for w_ in extra:
                    nop = nc.engines[ins.engine].nop(hint="wsplit", nofuse=True).ins
                    nop.sync_info = mybir.SyncInfo(on_wait=[w_], on_update=[])
                    for bb2 in nc.main_func.blocks:
                        if nop in bb2.instructions:
                            bb2.instructions.remove(nop)
                    carriers.append(nop)
                si.on_wait = keep
                for k, nop in enumerate(carriers):
                    insts.insert(i + k, nop)
                i += len(carriers)
            i += 1


# ---------------------------------------------------------------- device prog
def build_program(cfg, asm, bias_on):
    ntiles = cfg["ntiles"]
    n_cores = cfg["n_cores"]
    tpc = ntiles // n_cores
    npad = asm["npad"]
    cn = cfg["cn_tiles"]
    SEG = cfg["seg_rows"]
    nseg = asm["nseg"]
    ngroups = asm["ngroups"]
    group = asm["group"]
    L = asm["L"]
    Cgs = asm["Cgs"]
    chunk_tiles = asm["chunk_tiles"]
    f32 = mybir.dt.float32
    bf16 = mybir.dt.bfloat16
    fp16 = mybir.dt.float16
    i16 = mybir.dt.int16
    EQ = mybir.AluOpType.is_equal
    MUL = mybir.AluOpType.mult
    SUB = mybir.AluOpType.subtract

    tot16 = int((Cgs * P).sum()) // 16
    totC = int(Cgs.sum())

    nc = bass.Bass(num_swdge_queues=4)
    xT = nc.declare_dram_parameter("xT", [P, npad], bf16, isOutput=False)
    Wt = nc.declare_dram_parameter("Wt", [P, ELEMW], bf16, isOutput=False)
    degA_lo = nc.declare_dram_parameter("degA_lo", [P, ntiles], f32, isOutput=False)
    degA_hi = nc.declare_dram_parameter("degA_hi", [P, ntiles], f32, isOutput=False)
    iotaG = nc.declare_dram_parameter("iotaG", [P, group * P], fp16, isOutput=False)
    degB_lo = nc.declare_dram_parameter("degB_lo", [P, tpc], f32, isOutput=False)
    degB_hi = nc.declare_dram_parameter("degB_hi", [P, tpc], f32, isOutput=False)
    gidx = nc.declare_dram_parameter("gidx", [P, tot16], i16, isOutput=False)
    tloc = nc.declare_dram_parameter("tloc", [P, totC], fp16, isOutput=False)
    if bias_on:
        brow_p = nc.declare_dram_parameter("brow", [1, ELEMW], bf16, isOutput=False)
    outp = nc.declare_dram_parameter("outp", [tpc * P, D_OUT], f32, isOutput=True)
    g_table = nc.dram_tensor("g_table", [npad, ELEMW], fp16)

    nc.gpsimd.load_library(mlp)
    ng_regs = {}
    for s in range(nseg):
        C = int(Cgs[0, s])
        if C > 0:
            ng_regs[s] = nc.gpsimd.to_reg(C * P)

    with TileContext(nc) as tc:
        with (
            tc.tile_pool(name="const", bufs=1) as cpool,
            tc.tile_pool(name="xin", bufs=2) as xpool,
            tc.tile_pool(name="gw", bufs=2) as gwpool,
            tc.tile_pool(name="ps1", bufs=4, space="PSUM") as ps1,
            tc.tile_pool(name="gather", bufs=2) as gpool,
            tc.tile_pool(name="sel", bufs=3) as spool,
            tc.tile_pool(name="ps2", bufs=4, space="PSUM") as ps2,
            tc.tile_pool(name="outb", bufs=2) as opool,
        ):
            # ---- constants
            Wt_sb = cpool.tile([P, ELEMW], bf16)
            nc.sync.dma_start(out=Wt_sb[:], in_=Wt[:])
            iotaG_sb = cpool.tile([P, group * P], fp16)
            nc.sync.dma_start(out=iotaG_sb[:], in_=iotaG[:])
            if bias_on:
                ones_sb = cpool.tile([1, P], bf16)
                nc.vector.memset(ones_sb[:], 1.0)
                brow_sb = cpool.tile([1, ELEMW], bf16)
                nc.sync.dma_start(out=brow_sb[:], in_=brow_p[:])

            dAl = cpool.tile([P, ntiles], f32)
            dAh = cpool.tile([P, ntiles], f32)
            nc.sync.dma_start(out=dAl[:], in_=degA_lo[:])
            nc.sync.dma_start(out=dAh[:], in_=degA_hi[:])
            disA = cpool.tile([P, ntiles], f32)
            nc.vector.tensor_tensor(out=disA[:], in0=dAh[:], in1=dAl[:], op=SUB)
            nc.vector.reciprocal(out=disA[:], in_=disA[:])
            nc.scalar.activation(out=disA[:], in_=disA[:],
                                 func=mybir.ActivationFunctionType.Sqrt)

            dBl = cpool.tile([P, tpc], f32)
            dBh = cpool.tile([P, tpc], f32)
            nc.sync.dma_start(out=dBl[:], in_=degB_lo[:])
            nc.sync.dma_start(out=dBh[:], in_=degB_hi[:])
            disB = cpool.tile([P, tpc], f32)
            nc.vector.tensor_tensor(out=disB[:], in0=dBh[:], in1=dBl[:], op=SUB)
            nc.vector.reciprocal(out=disB[:], in_=disB[:])
            nc.scalar.activation(out=disB[:], in_=disB[:],
                                 func=mybir.ActivationFunctionType.Sqrt)

            gidx_sb = cpool.tile([P, tot16], i16)
            nc.sync.dma_start(out=gidx_sb[:], in_=gidx[:])
            tloc_sb = cpool.tile([P, totC], fp16)
            nc.sync.dma_start(out=tloc_sb[:], in_=tloc[:])

            # ---- phase 1: write g-table
            table_writes = []
            q = 0
            off = 0
            roff = 0
            for tcb in chunk_tiles:
                xt = xpool.tile([P, cn * P], bf16, tag="xt")
                nc.sync.dma_start(out=xt[:, :tcb * P], in_=xT[:, off:off + tcb * P])
                gbig = gwpool.tile([P, cn * ELEMW], fp16, tag="gbig")
                for j in range(tcb):
                    hp = ps1.tile([P, ELEMW], f32)
                    nc.tensor.matmul(out=hp[:], lhsT=xt[:, j * P:(j + 1) * P],
                                     rhs=Wt_sb[:], start=True, stop=not bias_on)
                    if bias_on:
                        nc.tensor.matmul(out=hp[:], lhsT=ones_sb[:], rhs=brow_sb[:],
                                         start=False, stop=True)
                    nc.vector.tensor_scalar(
                        out=gbig[:, j * ELEMW:(j + 1) * ELEMW], in0=hp[:],
                        scalar1=disA[:, q:q + 1], scalar2=None, op0=MUL)
                    q += 1
                wr = nc.sync.dma_start(out=g_table[roff:roff + tcb * P, :],
                                       in_=gbig[:, :tcb * ELEMW])
                table_writes.append(wr)
                off += tcb * P
                roff += tcb * P

            # ---- phase 2: segmented bulk gathers + one-hot matmul
            col16 = 0
            colC_base = np.zeros((ngroups, nseg), dtype=np.int64)
            acc = 0
            for g in range(ngroups):
                for s in range(nseg):
                    colC_base[g, s] = acc
                    acc += int(Cgs[g, s])

            orow = 0
            for g in range(ngroups):
                gts = {}
                for s in range(nseg):
                    C = int(Cgs[g, s])
                    if C == 0:
                        continue
                    NG = C * P
                    gb = gpool.tile([P, C, ELEMW], fp16, tag=f"gb{s}")
                    seg_base = s * SEG
                    seg_rows = min(SEG, npad - seg_base)
                    ga = nc.gpsimd.dma_gather(
                        gb[:, :, :], g_table[seg_base:seg_base + seg_rows, :],
                        gidx_sb[:, col16:col16 + NG // 16], NG, ng_regs[s],
                        ELEMW, single_packet=False,
                        queue_num=(g * nseg + s) % 4)
                    for w_ in table_writes:
                        add_dep_helper(ga.ins, w_.ins, reason="gather after table")
                    gts[s] = gb
                    col16 += NG // 16
                ob = opool.tile([P, group * D_OUT], f32, tag="ob")
                for t in range(group):
                    tl = g * group + t
                    mms = []   # (S_tile, k, s, c)
                    for s in range(nseg):
                        Lt = int(L[tl, s])
                        if Lt == 0:
                            continue
                        o0 = int(L[g * group:tl, s].sum())
                        c0 = o0 // P
                        c1 = (o0 + Lt - 1) // P
                        nsp = c1 - c0 + 1
                        S = spool.tile([P, nsp, P], fp16, tag="S")
                        cb = int(colC_base[g, s])
                        nc.vector.tensor_tensor(
                            out=S[:, :, :],
                            in0=iotaG_sb[:, t * P:(t + 1) * P][:, None, :]
                                .to_broadcast([P, nsp, P]),
                            in1=tloc_sb[:, cb + c0:cb + c1 + 1][:, :, None]
                                .to_broadcast([P, nsp, P]),
                            op=EQ)
                        for k, c in enumerate(range(c0, c1 + 1)):
                            mms.append((S, k, s, c))
                    pp = ps2.tile([P, D_OUT], f32)
                    if not mms:
                        nc.vector.memset(ob[:, t * D_OUT:(t + 1) * D_OUT], 0.0)
                        continue
                    for i, (S, k, s, c) in enumerate(mms):
                        nc.tensor.matmul(out=pp[:], lhsT=S[:, k, :],
                                         rhs=gts[s][:, c, 0:D_OUT],
                                         start=(i == 0), stop=(i == len(mms) - 1))
                    nc.vector.tensor_scalar(
                        out=ob[:, t * D_OUT:(t + 1) * D_OUT], in0=pp[:],
                        scalar1=disB[:, tl:tl + 1], scalar2=None, op0=MUL)
                nc.sync.dma_start(out=outp[orow:orow + group * P, :], in_=ob[:])
                orow += group * P

    split_excess_waits(nc)
    lower_extended_insts(nc)
    return nc


# ---------------------------------------------------------------- entry point
def kernel(x, W, b, edge_index):
    cfg = default_cfg()
    shared, per_core, brow, asm = preprocess(x, W, b, edge_index, cfg)
    bias_on = bool(np.any(np.asarray(b) != 0))
    nc = build_program(cfg, asm, bias_on)

    in_maps = []
    for c in range(cfg["n_cores"]):
        m = dict(shared)
        m.update(per_core[c])
        if bias_on:
            m["brow"] = brow
        in_maps.append(m)
    res = run_bass_kernel_spmd(nc, in_maps, list(range(cfg["n_cores"])))
    global LAST_RESULT
    LAST_RESULT = res

    return assemble(res.results, asm, cfg)


def assemble(results, asm, cfg):
    n_real = asm["n_real"]
    tpc = asm["tpc"]
    node_at = asm["node_at"]
    group = asm["group"]
    ngroups = asm["ngroups"]
    out = np.zeros((n_real, D_OUT), dtype=np.float32)
    for c in range(cfg["n_cores"]):
        r = results[c]["outp"]  # [tpc*P, 64]
        for g in range(ngroups):
            blk = r[g * group * P:(g + 1) * group * P].reshape(P, group, D_OUT)
            for t in range(group):
                nd = node_at[c * tpc + g * group + t]
                ok = nd >= 0
                out[nd[ok]] = blk[ok, t]
    return out
